# revision 71
# baseline (speedup 1.0000x reference)
"""Trainium2 Bass kernel for fused self-attention (nn_Attention).

Reference computes (only q is used; k/v inputs are dead):
    qkv = q @ in_w.T + qkv_bias ; qp,kp,vp = split(qkv)
    per head: softmax(qp @ kp.T / sqrt(hd)) @ vp
    net = concat_heads @ out_w.T + out_b

Sharding: tensor-parallel over heads. 16 heads / 8 cores = 2 heads/core.
Each core projects q against its 2-head slice of in_w, runs attention for
its (2 batch x 2 head) pairs, and computes a partial output projection
against its 128 columns of out_w. Host sums the 8 partials.

Cost-model-driven layout (matmul cost ~= out free size per accumulate
step; ACT cost ~= free size + fixed init):
  scores  [t, s] psum tiles [128, 2tt, 2h, 256s] (2 banks) -> one
          [128, 1024] exp per tile (128 exps total, the ACT floor)
  pv      out [s, e]: lhsT = exp slice [t, s128], rhs = V [t, 65]
          (64 dims + ones column -> denominator). N=65 per accumulate
          step: full PE efficiency, 2x cheaper than the [e, s] form.
          4 accumulators [128, 65] packed in ONE psum bank (start=True
          only on the bank's first matmul, stop=True only on the last;
          first write of each region replaces via pending-zero).
  norm    DVE reciprocal of denom col + per-partition tensor_scalar_mul
          -> attn [s, d] f16 (GPSIMD cannot touch PSUM, so every
          psum-reading element op lives on DVE)
  transp  PE-transpose [s, d] -> [d, s] (f16 psum), DVE copy to outT
  proj    lhsT = w2 slice, rhs = outT [d, s] -> partial [o, s]; DVE copy
          to f16 stage, DMA. The last 256 cols run as a short tail after
          the final exp with wide stage copies.
  qkv     Q/K bias via per-partition tensor_scalar_add on the psum->sbuf
          copy (no PE cost); V produced per t-tile in [t, vdim] layout
          (no PE transposes), V bias via a 1-row ones matmul.

Schedule: attention spine over (b, j-block of 256 tokens, t-pair).
pv runs one slot behind exp; normalize/transpose of block j ride the
first slots of block j+1. QKV projection and output projection are
deadline-scheduled into the spine's PE slack (weave), streaming against
the q-chunk DMA arrivals. Warmup matmuls hold the PE p-state ramp while
the first q chunk loads.
"""

import sys

for p in ("/opt/trn_rl_repo", "/root/.axon_site/_ro/trn_rl_repo"):
    if p not in sys.path:
        sys.path.append(p)

import numpy as np

B, S, D, H = 2, 2048, 1024, 16
BS = B * S  # 4096
HD = 64  # head dim
NCORES = 8
HPC = H // NCORES  # 2 heads per core -> 128 o-dims per core
JB = 8   # 256-token j-blocks per batch
PP = 8   # t-tile pairs per j-block

_COMPILED = {}
_TRUNC = None  # debug: emit only the first N (b, j) blocks
_SKIP = set()  # debug: {"exp", "pv", "weave"}


def _build():
    import concourse.bass as bass  # noqa: F401
    import concourse.mybir as mybir
    import concourse.tile as tile
    from concourse import bacc
    from concourse.masks import make_identity

    f16 = mybir.dt.float16
    f32 = mybir.dt.float32
    AF = mybir.ActivationFunctionType

    nc = bacc.Bacc("TRN2", target_bir_lowering=False, debug=False,
                   num_devices=NCORES)

    # weight params are host-prearranged to [128, 8, x] so every DMA
    # descriptor is a contiguous >=2KB per-partition run
    qT_d = nc.declare_dram_parameter("qT", [D, BS], f16, isOutput=False)
    wq_d = nc.declare_dram_parameter("wq", [128, 8, 128], f16, isOutput=False)
    wk_d = nc.declare_dram_parameter("wk", [128, 8, 128], f16, isOutput=False)
    wv_d = nc.declare_dram_parameter("wv", [128, 8, 128], f16, isOutput=False)
    w2_d = nc.declare_dram_parameter("w2", [128, D], f16, isOutput=False)
    qkb_d = nc.declare_dram_parameter("qkb", [128, 2], f32, isOutput=False)
    vb_d = nc.declare_dram_parameter("vb", [1, 128], f16, isOutput=False)
    out_d = nc.declare_dram_parameter("partial", [D, BS], f16, isOutput=True)

    with tile.TileContext(nc) as tc:
        with (
            tc.tile_pool(name="persist", bufs=1) as persist,
            tc.tile_pool(name="exp", bufs=4) as exp_pool,
            tc.tile_pool(name="attn", bufs=2) as attn_pool,
            tc.tile_pool(name="recip", bufs=2) as recip_pool,
            tc.tile_pool(name="stage", bufs=4) as stage_pool,
            tc.tile_pool(name="sc", bufs=2, space="PSUM") as sc_pool,
            tc.tile_pool(name="pv", bufs=2, space="PSUM") as pv_pool,
            tc.tile_pool(name="qkps", bufs=1, space="PSUM") as qk_ps,
            tc.tile_pool(name="wvps", bufs=1, space="PSUM") as wv_ps,
        ):
            # ---- resident SBUF tensors ----
            q_sb = persist.tile([128, 8, BS], f16)      # 64KB/part
            wq_sb = persist.tile([128, 8, 128], f16)
            wk_sb = persist.tile([128, 8, 128], f16)
            wv_sb = persist.tile([128, 8, 128], f16)
            w2_sb = persist.tile([128, D], f16)
            qkb_sb = persist.tile([128, 2], f32)
            vb_sb = persist.tile([1, 128], f16)
            ones_sb = persist.tile([1, 128], f16)
            # Matmuls with partition-offset operands break on real HW
            # when the psum output is not bank-aligned, so score operands
            # must be full-partition: K keeps both heads' dims stacked
            # (lhsT [128, t]), and Q is stored in two zero-padded planes
            # ([Q_h0; 0] and [0; Q_h1]) so each head's scores come from a
            # full-partition rhs -- the pad rows contribute zero
            qpad_sb = persist.tile([128, 2, BS], f16)   # [dim|pad, head, b*s]
            k_sb = persist.tile([128, BS], f16)
            v_sb = persist.tile([128, B, 16, 130], f16)  # [t, b, tile, dims]
            outT_sb = persist.tile([128, B, 2048], f16)  # [d, b, s]
            ident_sb = persist.tile([128, 128], f16)
            warm_sb = persist.tile([1, 8], f32)
            wlhs_sb = persist.tile([128, 128], f16)

            # warmup lhsT first: one memset, then the PE warmup stream can
            # start immediately and hold the p-state ramp
            nc.vector.memset(wlhs_sb[:, :], 0.0)
            make_identity(nc, ident_sb[:, :])
            # force the exp ACT-table load before DMAs occupy the queues
            nc.vector.memset(warm_sb[:, :], 0.0)
            nc.scalar.activation(warm_sb[:, :], warm_sb[:, :], AF.Exp)
            nc.vector.memset(ones_sb[:, :], 1.0)
            # ones columns of v_sb (64: h0 denom, 129: h1 denom) are set
            # once; per-tile V copies never overwrite them
            nc.vector.memset(v_sb[:, :, :, 64:65], 1.0)
            nc.vector.memset(v_sb[:, :, :, 129:130], 1.0)
            # zero pad-halves of the Q planes (never written afterwards);
            # on Pool (SBUF-only op) so DVE stays free for the first Q/K
            # projection copies
            nc.gpsimd.memset(qpad_sb[64:128, 0, :], 0.0)
            nc.gpsimd.memset(qpad_sb[0:64, 1, :], 0.0)

            # loads ordered by first use; q chunk 0 split in halves so the
            # first attention block can start earlier
            qT_t = qT_d.rearrange("(n p) m -> p n m", p=128)
            nc.sync.dma_start(qkb_sb[:, :], qkb_d[:, :])
            nc.sync.dma_start(vb_sb[:, :], vb_d[:, :])
            nc.sync.dma_start(wq_sb[:, :, :], wq_d[:, :, :])
            nc.sync.dma_start(q_sb[:, :, 0:256], qT_t[:, :, 0:256])
            nc.sync.dma_start(wk_sb[:, :, :], wk_d[:, :, :])
            nc.sync.dma_start(wv_sb[:, :, :], wv_d[:, :, :])
            nc.sync.dma_start(q_sb[:, :, 256:512], qT_t[:, :, 256:512])
            for scc in range(1, 4):
                nc.sync.dma_start(
                    q_sb[:, :, scc * 512:(scc + 1) * 512],
                    qT_t[:, :, scc * 512:(scc + 1) * 512])
            nc.sync.dma_start(w2_sb[:, :], w2_d[:, :])
            for scc in range(4, 8):
                nc.sync.dma_start(
                    q_sb[:, :, scc * 512:(scc + 1) * 512],
                    qT_t[:, :, scc * 512:(scc + 1) * 512])

            # ---- work-unit emitters (atomic closures) ------------------
            uid = [0]

            def qj_unit(b, j, pool=None):
                """Q projection for one 256-token j-block -> two
                zero-padded qpad planes (same-partition copies only)"""
                def emit():
                    uid[0] += 1
                    s0 = b * 2048 + j * 256
                    ps = (pool or qk_ps).tile([128, 256], f32,
                                              tag="wv" if pool else "qk",
                                              name=f"q{uid[0]}")
                    for dk in range(8):
                        nc.tensor.matmul(
                            ps[:, :], wq_sb[:, dk, :],
                            q_sb[:, dk, s0:s0 + 256],
                            start=(dk == 0), stop=(dk == 7))
                    nc.vector.tensor_scalar_add(
                        qpad_sb[0:64, 0, s0:s0 + 256], ps[0:64, :],
                        qkb_sb[0:64, 0:1])
                    nc.vector.tensor_scalar_add(
                        qpad_sb[64:128, 1, s0:s0 + 256], ps[64:128, :],
                        qkb_sb[64:128, 0:1])
                return emit

            def k_unit(b, pp):
                """K projection for one t-pair (256 tokens) -> k_sb"""
                def emit():
                    uid[0] += 1
                    t0 = b * 2048 + pp * 256
                    ps = qk_ps.tile([128, 256], f32, tag="qk",
                                    name=f"k{uid[0]}")
                    for dk in range(8):
                        nc.tensor.matmul(
                            ps[:, :], wk_sb[:, dk, :],
                            q_sb[:, dk, t0:t0 + 256],
                            start=(dk == 0), stop=(dk == 7))
                    nc.vector.tensor_scalar_add(
                        k_sb[:, t0:t0 + 256], ps[:, :], qkb_sb[:, 1:2])
                return emit

            def v_unit(b, st):
                """V projection for one t-tile in [t, vdim] layout."""
                def emit():
                    uid[0] += 1
                    t0 = b * 2048 + st * 128
                    ps = wv_ps.tile([128, 128], f32, tag="wv",
                                    name=f"v{uid[0]}")
                    for dk in range(8):
                        nc.tensor.matmul(
                            ps[:, :], q_sb[:, dk, t0:t0 + 128],
                            wv_sb[:, dk, :],
                            start=(dk == 0), stop=False)
                    nc.tensor.matmul(  # += ones.T @ vb  (per-vdim bias)
                        ps[:, :], ones_sb[0:1, :], vb_sb[0:1, :],
                        start=False, stop=True)
                    # GPSIMD cannot access PSUM (BIR verifier); one
                    # strided DVE copy fills both head halves around the
                    # resident ones columns
                    nc.vector.tensor_copy(
                        v_sb[:, b, st, 0:130].rearrange(
                            "p (g c) -> p g c", g=2)[:, :, 0:64],
                        ps[:, :].rearrange("p (g c) -> p g c", g=2))
                return emit

            def proj_part(b, ot, off, w, eng=None):
                """partial[ot*128:, b*2048+off : +w] via stage copy."""
                def emit():
                    uid[0] += 1
                    ps = wv_ps.tile([128, w], f32, tag="wv",
                                    name=f"p{uid[0]}")
                    nc.tensor.matmul(
                        ps[:, :], w2_sb[:, ot * 128:(ot + 1) * 128],
                        outT_sb[:, b, off:off + w], start=True, stop=True)
                    stg = stage_pool.tile([128, w], f16, tag="st",
                                          name=f"s{uid[0]}")
                    (eng or nc.vector).tensor_copy(stg[:, :], ps[:, :])
                    nc.sync.dma_start(
                        out_d[ot * 128:(ot + 1) * 128,
                              b * 2048 + off:b * 2048 + off + w],
                        stg[:, :])
                return emit

            # ---- spine machinery ---------------------------------------
            pv_state = {}

            def emit_pv(e, b, j, p):
                st = pv_state[(b, j)]
                for sub in range(2):
                    step = 2 * p + sub
                    for k in range(2):
                        for h in range(2):
                            nc.tensor.matmul(
                                st["tile"][:, 2 * k + h, :],
                                e[:, sub, h, k * 128:(k + 1) * 128],
                                v_sb[:, b, 2 * p + sub, 65 * h:65 * h + 65],
                                start=(step == 0 and k == 0 and h == 0),
                                stop=(step == 15 and k == 1 and h == 1))

            def norm_block(b, j, split=False):
                def emit():
                    pvt = pv_state[(b, j)]["tile"]
                    rc = recip_pool.tile([128, 4, 1], f32, tag="rc",
                                         name=f"rc{b}_{j}")
                    nc.vector.reciprocal(rc[:, :, :], pvt[:, :, 64:65])
                    if split:
                        # tail only: k-subtiles in SEPARATE tiles so the
                        # DVE and ACT writers don't serialize on a tile WAW;
                        # ACT Copy with per-partition scale = normalize
                        ats = [attn_pool.tile([128, 1, 128], f16, tag="at",
                                              name=f"at{b}_{j}_{k}")
                               for k in range(2)]
                        pv_state[(b, j)]["attn"] = ats
                        for k in range(2):
                            for h in range(2):
                                if k == 1:
                                    nc.scalar.activation(
                                        ats[k][:, 0, 64 * h:64 * h + 64],
                                        pvt[:, 2 * k + h, 0:64],
                                        AF.Copy, scale=rc[:, 2 * k + h, 0:1])
                                else:
                                    nc.vector.tensor_scalar_mul(
                                        ats[k][:, 0, 64 * h:64 * h + 64],
                                        pvt[:, 2 * k + h, 0:64],
                                        rc[:, 2 * k + h, 0:1])
                        return
                    at = attn_pool.tile([128, 2, 128], f16, tag="at",
                                        name=f"at{b}_{j}")
                    pv_state[(b, j)]["attn"] = at
                    for k in range(2):
                        for h in range(2):
                            nc.vector.tensor_scalar_mul(
                                at[:, k, 64 * h:64 * h + 64],
                                pvt[:, 2 * k + h, 0:64],
                                rc[:, 2 * k + h, 0:1])
                return emit

            def transp_block(b, j, split=False):
                def emit():
                    at = pv_state[(b, j)]["attn"]
                    for k in range(2):
                        uid[0] += 1
                        src = at[k][:, 0, :] if split else at[:, k, :]
                        tr = wv_ps.tile([128, 128], f16, tag="wv",
                                        name=f"tr{uid[0]}")
                        nc.tensor.transpose(tr[:, :], src, ident_sb[:, :])
                        dst = outT_sb[:, b, j * 256 + k * 128:
                                      j * 256 + (k + 1) * 128]
                        if split and k == 1:
                            nc.scalar.activation(dst, tr[:, :], AF.Copy)
                        else:
                            nc.vector.tensor_copy(dst, tr[:, :])
                return emit

            # ---- weave schedule ---------------------------------------
            sched = {}

            def at(b, j, p, *units):
                sched.setdefault((b, j, p), []).extend(units)

            # b0 j0: stream b0's K and V against q-chunk arrivals; K units
            # lead their slot (scores are the critical consumer)
            at(0, 0, 0, k_unit(0, 1), v_unit(0, 0), v_unit(0, 1))
            at(0, 0, 1, k_unit(0, 2), k_unit(0, 3), v_unit(0, 2))
            at(0, 0, 2, qj_unit(0, 1), v_unit(0, 3), v_unit(0, 4))
            at(0, 0, 3, k_unit(0, 4), k_unit(0, 5), v_unit(0, 5))
            at(0, 0, 4, v_unit(0, 6), v_unit(0, 7), v_unit(0, 8))
            at(0, 0, 5, k_unit(0, 6), k_unit(0, 7), v_unit(0, 9))
            at(0, 0, 6, v_unit(0, 10), v_unit(0, 11), qj_unit(0, 2))
            at(0, 0, 7, v_unit(0, 12), v_unit(0, 13))
            # b0 j1..j7: rest of Q(b0), all QKV(b1) (placed after their
            # q-chunk DMA arrivals), proj(b0) filling the chunk-wait slack
            at(0, 1, 0, v_unit(0, 14))
            at(0, 1, 1, v_unit(0, 15))
            at(0, 1, 2, qj_unit(0, 3))
            at(0, 1, 3, v_unit(1, 0))
            at(0, 1, 4, k_unit(1, 0))
            at(0, 1, 5, v_unit(1, 1))
            at(0, 1, 6, k_unit(1, 1))
            at(0, 1, 7, qj_unit(0, 4))
            at(0, 2, 0, v_unit(1, 2))
            at(0, 2, 1, v_unit(1, 3))
            at(0, 2, 2, k_unit(1, 2))
            at(0, 2, 3, proj_part(0, 0, 0, 512), v_unit(1, 4))
            at(0, 2, 4, proj_part(0, 1, 0, 512))
            at(0, 2, 5, k_unit(1, 3))
            at(0, 2, 6, v_unit(1, 5))
            at(0, 2, 7, v_unit(1, 6))
            at(0, 3, 0, v_unit(1, 7))
            at(0, 3, 1, proj_part(0, 2, 0, 512))
            at(0, 3, 2, k_unit(1, 4))
            at(0, 3, 3, proj_part(0, 3, 0, 512))
            at(0, 3, 4, proj_part(0, 4, 0, 512))
            at(0, 3, 5, v_unit(1, 8))
            at(0, 3, 6, proj_part(0, 5, 0, 512))
            at(0, 3, 7, proj_part(0, 6, 0, 512))
            at(0, 4, 0, v_unit(1, 9))
            at(0, 4, 1, qj_unit(0, 5))
            at(0, 4, 2, k_unit(1, 5))
            at(0, 4, 3, v_unit(1, 10))
            at(0, 4, 4, proj_part(0, 7, 0, 512))
            at(0, 4, 5, proj_part(0, 0, 512, 512))
            at(0, 4, 6, v_unit(1, 11))
            at(0, 4, 7, k_unit(1, 6))
            at(0, 5, 0, v_unit(1, 12))
            at(0, 5, 1, proj_part(0, 1, 512, 512))
            at(0, 5, 2, v_unit(1, 13))
            at(0, 5, 3, proj_part(0, 2, 512, 512))
            at(0, 5, 4, qj_unit(0, 6))
            at(0, 5, 5, k_unit(1, 7))
            at(0, 5, 6, v_unit(1, 14))
            at(0, 5, 7, proj_part(0, 3, 512, 512))
            at(0, 6, 0, qj_unit(0, 7))
            at(0, 6, 1, v_unit(1, 15))
            at(0, 6, 2, proj_part(0, 4, 512, 512))
            at(0, 6, 3, proj_part(0, 5, 512, 512))
            at(0, 6, 4, qj_unit(1, 0))
            at(0, 6, 5, proj_part(0, 6, 512, 512))
            at(0, 6, 6, qj_unit(1, 1))
            at(0, 6, 7, proj_part(0, 7, 512, 512))
            at(0, 7, 0, qj_unit(1, 2))
            at(0, 7, 1, proj_part(0, 0, 1024, 512))
            at(0, 7, 2, proj_part(0, 1, 1024, 512))
            at(0, 7, 3, proj_part(0, 2, 1024, 512))
            at(0, 7, 4, proj_part(0, 3, 1024, 512))
            at(0, 7, 5, proj_part(0, 4, 1024, 512))
            at(0, 7, 6, proj_part(0, 5, 1024, 512))
            at(0, 7, 7, qj_unit(1, 3))
            # b1: rest of Q(b1), proj(b0 tail), all proj(b1)
            at(1, 0, 0, proj_part(0, 6, 1024, 512))
            at(1, 0, 1, proj_part(0, 7, 1024, 512))
            at(1, 0, 4, qj_unit(1, 4))
            at(1, 0, 5, proj_part(0, 0, 1536, 512))
            at(1, 0, 6, proj_part(0, 1, 1536, 512))
            at(1, 0, 7, proj_part(0, 2, 1536, 512))
            at(1, 1, 0, proj_part(0, 3, 1536, 512))
            at(1, 1, 1, proj_part(0, 4, 1536, 512))
            at(1, 1, 2, proj_part(0, 5, 1536, 512))
            at(1, 1, 3, proj_part(0, 6, 1536, 512))
            at(1, 1, 4, proj_part(0, 7, 1536, 512))
            at(1, 1, 5, qj_unit(1, 5))
            at(1, 2, 0, qj_unit(1, 6))
            at(1, 2, 1, qj_unit(1, 7))
            for ot in range(8):
                at(1, 3, ot, proj_part(1, ot, 0, 512))
            for ot in range(8):
                at(1, 5, ot, proj_part(1, ot, 512, 512))
            for i, ot in enumerate(range(5)):
                at(1, 6, 3 + i, proj_part(1, ot, 1024, 512))
            at(1, 7, 0, proj_part(1, 5, 1024, 512))
            at(1, 7, 1, proj_part(1, 6, 1024, 512))
            at(1, 7, 2, proj_part(1, 7, 1024, 512))
            at(1, 7, 2, proj_part(1, 0, 1536, 256),
               proj_part(1, 1, 1536, 256))
            at(1, 7, 3, proj_part(1, 2, 1536, 256),
               proj_part(1, 3, 1536, 256),
               proj_part(1, 4, 1536, 256))
            at(1, 7, 4, proj_part(1, 5, 1536, 256),
               proj_part(1, 6, 1536, 256),
               proj_part(1, 7, 1536, 256))

            deferred = {}  # (b, j, p) -> [callables]

            # ---- phase 1: warmup + minimal pre-work --------------------
            wps = wv_ps.tile([128, 128], f32, tag="wv", name="wps")
            for i in range(37):
                nc.tensor.matmul(wps[:, :], wlhs_sb[:, :], wlhs_sb[:, :],
                                 start=True, stop=True)
            # pre-phase Q goes through the wv_ps bank so K's matmuls don't
            # serialize behind Q's psum->sbuf copy (qk_ps is single-buffered)
            qj_unit(0, 0, pool=wv_ps)()
            k_unit(0, 0)()

            # ---- attention spine ---------------------------------------
            # pv runs TWO slots behind exp so a block's last pv never
            # catches up with its exp at the j boundary
            e_queue = []
            nblocks = 0
            for b in range(B):
                for j in range(JB):
                    if _TRUNC is not None and nblocks >= _TRUNC:
                        continue
                    nblocks += 1
                    pv_state[(b, j)] = {
                        "tile": pv_pool.tile([128, 4, 65], f32, tag="pv",
                                             name=f"pv{b}_{j}")}
                    for p in range(PP):
                        s0 = b * 2048 + j * 256
                        sc = sc_pool.tile([128, 2, 2, 256], f32, tag="sc",
                                          name=f"sc{b}_{j}_{p}")
                        if "sc" not in _SKIP:
                            # one accumulation group per bank: h0 starts
                            # (pending-zeroes the bank), h1's first write
                            # replaces its half and stops the group
                            for sub in range(2):
                                t0 = b * 2048 + (2 * p + sub) * 128
                                for h in range(2):
                                    nc.tensor.matmul(
                                        sc[:, sub, h, :],
                                        k_sb[:, t0:t0 + 128],
                                        qpad_sb[:, h, s0:s0 + 256],
                                        start=(h == 0), stop=(h == 1))
                        for u in deferred.pop((b, j, p), ()):
                            u()
                        if "weave" not in _SKIP:
                            for u in sched.pop((b, j, p), ()):
                                u()
                        if len(e_queue) >= 2:
                            emit_pv(*e_queue.pop(0))
                        if (b, j) == (1, 7) and p == 7 and e_queue:
                            emit_pv(*e_queue.pop(0))
                        if "exp" in _SKIP:
                            continue
                        e = exp_pool.tile([128, 2, 2, 256], f16, tag="e",
                                          name=f"e{b}_{j}_{p}")
                        nc.scalar.activation(e[:, :, :, :], sc[:, :, :, :],
                                             AF.Exp, scale=0.125)
                        if "pv" not in _SKIP:
                            e_queue.append((e, b, j, p))
                    # norm/transpose of j ride block j+1's slots 2/3
                    # (pv(j, p7) lands at slot 1 via the lag-2 queue)
                    if (b, j) != (1, 7):
                        nb, nj = (b, j + 1) if j < 7 else (b + 1, 0)
                        # the final block runs at lag 1, so (1,6)'s pv
                        # flushes one slot earlier and its norm/transpose
                        # compress forward one slot
                        off = 1 if (b, j) == (1, 6) else 2
                        deferred.setdefault((nb, nj, off), []).append(
                            norm_block(b, j))
                        deferred.setdefault((nb, nj, off + 1), []).append(
                            transp_block(b, j))

            if _TRUNC is not None:
                # debug truncation: drain pv chains, skip norm/tail, dump
                # qk_sb and v_sb for host inspection
                for item in e_queue:
                    emit_pv(*item)
                e_queue = []
                nc.sync.dma_start(out_d[0:128, :], qpad_sb[:, 0, :])
                nc.sync.dma_start(out_d[128:256, :], k_sb[:, :])
                nc.sync.dma_start(
                    out_d[256:384, 0:16 * 130],
                    v_sb[:, 0, :, :])
                nc.sync.dma_start(out_d[384:512, :], q_sb[:, 0, :])
                nc.sync.dma_start(out_d[512:640, 0:1024],
                                  wq_sb[:, :, :])
            if _TRUNC is None:
                assert not sched, f"unconsumed weave slots: {list(sched)}"

                # ---- tail: drain the lag queue, norm/transpose j7, project
                # the final 256 cols with wide stage copies ----
                for item in e_queue:
                    emit_pv(*item)
                norm_block(1, 7, split=True)()
                transp_block(1, 7, split=True)()
                out_t = out_d.rearrange("(a p) m -> p a m", p=128)
                for half in range(2):
                    tps = sc_pool.tile([128, 2, 2, 256], f32, tag="sc",
                                       name=f"tail{half}")
                    for i in range(4):
                        ot = half * 4 + i
                        nc.tensor.matmul(
                            tps[:, i // 2, i % 2, :],
                            w2_sb[:, ot * 128:(ot + 1) * 128],
                            outT_sb[:, 1, 1792:2048], start=True, stop=True)
                    stg = stage_pool.tile([128, 4, 256], f16, tag="tt",
                                          name=f"tstg{half}")
                    if half == 0:
                        nc.vector.tensor_copy(stg[:, :, :], tps[:, :, :, :])
                    else:
                        nc.scalar.activation(stg[:, :, :], tps[:, :, :, :],
                                             AF.Copy)
                    nc.sync.dma_start(
                        out_t[:, half * 4:(half + 1) * 4, 3840:4096],
                        stg[:, :, :])
    nc.compile()
    return nc


def _get_nc():
    if "nc" not in _COMPILED:
        _COMPILED["nc"] = _build()
    return _COMPILED["nc"]


def _prep_inputs(q, in_w, qkv_bias):
    f16 = np.float16
    qT = np.ascontiguousarray(q.transpose(2, 0, 1).reshape(D, BS)).astype(f16)
    def pre(w):  # [128 odims, 1024 d] -> [128 part, 8 dk, 128 odims]
        return np.ascontiguousarray(
            w.T.reshape(8, 128, 128).transpose(1, 0, 2)).astype(f16)

    maps = []
    for c in range(NCORES):
        r = slice(128 * c, 128 * (c + 1))
        wq, wk, wv = in_w[0:D][r], in_w[D:2 * D][r], in_w[2 * D:3 * D][r]
        maps.append({
            "qT": qT,
            "wq": pre(wq),
            "wk": pre(wk),
            "wv": pre(wv),
            "w2": None,  # filled with out_w slice
            "qkb": np.ascontiguousarray(
                np.stack([qkv_bias[0:D][r], qkv_bias[D:2 * D][r]], axis=1)
            ).astype(np.float32),
            "vb": np.ascontiguousarray(
                qkv_bias[2 * D:3 * D][r][None, :]).astype(f16),
        })
    return maps


def kernel(q, k, v, in_w, qkv_bias, out_w, out_b, _trace=False):
    from concourse.bass_utils import run_bass_kernel_spmd

    q = np.asarray(q, dtype=np.float32)
    in_w = np.asarray(in_w, dtype=np.float32)
    qkv_bias = np.asarray(qkv_bias, dtype=np.float32)
    out_w = np.asarray(out_w, dtype=np.float32)
    out_b = np.asarray(out_b, dtype=np.float32)

    nc = _get_nc()
    in_maps = _prep_inputs(q, in_w, qkv_bias)
    for c in range(NCORES):
        r = slice(128 * c, 128 * (c + 1))
        in_maps[c]["w2"] = np.ascontiguousarray(out_w[:, r].T).astype(np.float16)

    res = run_bass_kernel_spmd(
        nc, in_maps, core_ids=list(range(NCORES)), trace=_trace,
    )
    total = np.zeros((D, BS), dtype=np.float32)
    for c in range(NCORES):
        total += res.results[c]["partial"].astype(np.float32)
    net = total.T + out_b[None, :]
    out = net.reshape(B, S, D).astype(np.float32)
    if _trace:
        return out, res
    return out


# revision 72
# speedup vs baseline: 1.0003x; 1.0003x over previous
"""Trainium2 Bass kernel for fused self-attention (nn_Attention).

Reference computes (only q is used; k/v inputs are dead):
    qkv = q @ in_w.T + qkv_bias ; qp,kp,vp = split(qkv)
    per head: softmax(qp @ kp.T / sqrt(hd)) @ vp
    net = concat_heads @ out_w.T + out_b

Sharding: tensor-parallel over heads. 16 heads / 8 cores = 2 heads/core.
Each core projects q against its 2-head slice of in_w, runs attention for
its (2 batch x 2 head) pairs, and computes a partial output projection
against its 128 columns of out_w. Host sums the 8 partials.

Cost-model-driven layout (matmul cost ~= out free size per accumulate
step; ACT cost ~= free size + fixed init):
  scores  [t, s] psum tiles [128, 2tt, 2h, 256s] (2 banks) -> one
          [128, 1024] exp per tile (128 exps total, the ACT floor)
  pv      out [s, e]: lhsT = exp slice [t, s128], rhs = V [t, 65]
          (64 dims + ones column -> denominator). N=65 per accumulate
          step: full PE efficiency, 2x cheaper than the [e, s] form.
          4 accumulators [128, 65] packed in ONE psum bank (start=True
          only on the bank's first matmul, stop=True only on the last;
          first write of each region replaces via pending-zero).
  norm    DVE reciprocal of denom col + per-partition tensor_scalar_mul
          -> attn [s, d] f16 (GPSIMD cannot touch PSUM, so every
          psum-reading element op lives on DVE)
  transp  PE-transpose [s, d] -> [d, s] (f16 psum), DVE copy to outT
  proj    lhsT = w2 slice, rhs = outT [d, s] -> partial [o, s]; DVE copy
          to f16 stage, DMA. The last 256 cols run as a short tail after
          the final exp with wide stage copies.
  qkv     Q/K bias via per-partition tensor_scalar_add on the psum->sbuf
          copy (no PE cost); V produced per t-tile in [t, vdim] layout
          (no PE transposes), V bias via a 1-row ones matmul.

Schedule: attention spine over (b, j-block of 256 tokens, t-pair).
pv runs one slot behind exp; normalize/transpose of block j ride the
first slots of block j+1. QKV projection and output projection are
deadline-scheduled into the spine's PE slack (weave), streaming against
the q-chunk DMA arrivals. Warmup matmuls hold the PE p-state ramp while
the first q chunk loads.
"""

import sys

for p in ("/opt/trn_rl_repo", "/root/.axon_site/_ro/trn_rl_repo"):
    if p not in sys.path:
        sys.path.append(p)

import numpy as np

B, S, D, H = 2, 2048, 1024, 16
BS = B * S  # 4096
HD = 64  # head dim
NCORES = 8
HPC = H // NCORES  # 2 heads per core -> 128 o-dims per core
JB = 8   # 256-token j-blocks per batch
PP = 8   # t-tile pairs per j-block

_COMPILED = {}
_TRUNC = None  # debug: emit only the first N (b, j) blocks
_SKIP = set()  # debug: {"exp", "pv", "weave"}


def _build():
    import concourse.bass as bass  # noqa: F401
    import concourse.mybir as mybir
    import concourse.tile as tile
    from concourse import bacc
    from concourse.masks import make_identity

    f16 = mybir.dt.float16
    f32 = mybir.dt.float32
    AF = mybir.ActivationFunctionType

    nc = bacc.Bacc("TRN2", target_bir_lowering=False, debug=False,
                   num_devices=NCORES)

    # weight params are host-prearranged to [128, 8, x] so every DMA
    # descriptor is a contiguous >=2KB per-partition run
    qT_d = nc.declare_dram_parameter("qT", [D, BS], f16, isOutput=False)
    wq_d = nc.declare_dram_parameter("wq", [128, 8, 128], f16, isOutput=False)
    wk_d = nc.declare_dram_parameter("wk", [128, 8, 128], f16, isOutput=False)
    wv_d = nc.declare_dram_parameter("wv", [128, 8, 128], f16, isOutput=False)
    w2_d = nc.declare_dram_parameter("w2", [128, D], f16, isOutput=False)
    qkb_d = nc.declare_dram_parameter("qkb", [128, 2], f32, isOutput=False)
    vb_d = nc.declare_dram_parameter("vb", [1, 128], f16, isOutput=False)
    out_d = nc.declare_dram_parameter("partial", [D, BS], f16, isOutput=True)

    with tile.TileContext(nc) as tc:
        with (
            tc.tile_pool(name="persist", bufs=1) as persist,
            tc.tile_pool(name="exp", bufs=4) as exp_pool,
            tc.tile_pool(name="attn", bufs=2) as attn_pool,
            tc.tile_pool(name="recip", bufs=2) as recip_pool,
            tc.tile_pool(name="stage", bufs=4) as stage_pool,
            tc.tile_pool(name="sc", bufs=2, space="PSUM") as sc_pool,
            tc.tile_pool(name="pv", bufs=2, space="PSUM") as pv_pool,
            tc.tile_pool(name="qkps", bufs=1, space="PSUM") as qk_ps,
            tc.tile_pool(name="wvps", bufs=1, space="PSUM") as wv_ps,
        ):
            # ---- resident SBUF tensors ----
            q_sb = persist.tile([128, 8, BS], f16)      # 64KB/part
            wq_sb = persist.tile([128, 8, 128], f16)
            wk_sb = persist.tile([128, 8, 128], f16)
            wv_sb = persist.tile([128, 8, 128], f16)
            w2_sb = persist.tile([128, D], f16)
            qkb_sb = persist.tile([128, 2], f32)
            vb_sb = persist.tile([1, 128], f16)
            ones_sb = persist.tile([1, 128], f16)
            # Matmuls with partition-offset operands break on real HW
            # when the psum output is not bank-aligned, so score operands
            # must be full-partition: K keeps both heads' dims stacked
            # (lhsT [128, t]), and Q is stored in two zero-padded planes
            # ([Q_h0; 0] and [0; Q_h1]) so each head's scores come from a
            # full-partition rhs -- the pad rows contribute zero
            qpad_sb = persist.tile([128, 2, BS], f16)   # [dim|pad, head, b*s]
            k_sb = persist.tile([128, BS], f16)
            v_sb = persist.tile([128, B, 16, 130], f16)  # [t, b, tile, dims]
            outT_sb = persist.tile([128, B, 2048], f16)  # [d, b, s]
            ident_sb = persist.tile([128, 128], f16)
            warm_sb = persist.tile([1, 8], f32)
            wlhs_sb = persist.tile([128, 128], f16)

            # warmup lhsT first: one memset, then the PE warmup stream can
            # start immediately and hold the p-state ramp
            nc.vector.memset(wlhs_sb[:, :], 0.0)
            make_identity(nc, ident_sb[:, :])
            # force the exp ACT-table load before DMAs occupy the queues
            nc.vector.memset(warm_sb[:, :], 0.0)
            nc.scalar.activation(warm_sb[:, :], warm_sb[:, :], AF.Exp)
            nc.vector.memset(ones_sb[:, :], 1.0)
            # ones columns of v_sb (64: h0 denom, 129: h1 denom) are set
            # once; per-tile V copies never overwrite them
            nc.vector.memset(v_sb[:, :, :, 64:65], 1.0)
            nc.vector.memset(v_sb[:, :, :, 129:130], 1.0)
            # zero pad-halves of the Q planes (never written afterwards);
            # on Pool (SBUF-only op) so DVE stays free for the first Q/K
            # projection copies
            nc.gpsimd.memset(qpad_sb[64:128, 0, :], 0.0)
            nc.gpsimd.memset(qpad_sb[0:64, 1, :], 0.0)

            # loads ordered by first use; q chunk 0 split in halves so the
            # first attention block can start earlier
            qT_t = qT_d.rearrange("(n p) m -> p n m", p=128)
            nc.sync.dma_start(qkb_sb[:, :], qkb_d[:, :])
            nc.sync.dma_start(vb_sb[:, :], vb_d[:, :])
            nc.sync.dma_start(wq_sb[:, :, :], wq_d[:, :, :])
            nc.sync.dma_start(q_sb[:, :, 0:256], qT_t[:, :, 0:256])
            nc.sync.dma_start(wk_sb[:, :, :], wk_d[:, :, :])
            nc.sync.dma_start(wv_sb[:, :, :], wv_d[:, :, :])
            nc.sync.dma_start(q_sb[:, :, 256:512], qT_t[:, :, 256:512])
            for scc in range(1, 4):
                nc.sync.dma_start(
                    q_sb[:, :, scc * 512:(scc + 1) * 512],
                    qT_t[:, :, scc * 512:(scc + 1) * 512])
            nc.sync.dma_start(w2_sb[:, :], w2_d[:, :])
            for scc in range(4, 8):
                nc.sync.dma_start(
                    q_sb[:, :, scc * 512:(scc + 1) * 512],
                    qT_t[:, :, scc * 512:(scc + 1) * 512])

            # ---- work-unit emitters (atomic closures) ------------------
            uid = [0]

            def qj_unit(b, j, pool=None):
                """Q projection for one 256-token j-block -> two
                zero-padded qpad planes (same-partition copies only)"""
                def emit():
                    uid[0] += 1
                    s0 = b * 2048 + j * 256
                    ps = (pool or qk_ps).tile([128, 256], f32,
                                              tag="wv" if pool else "qk",
                                              name=f"q{uid[0]}")
                    for dk in range(8):
                        nc.tensor.matmul(
                            ps[:, :], wq_sb[:, dk, :],
                            q_sb[:, dk, s0:s0 + 256],
                            start=(dk == 0), stop=(dk == 7))
                    nc.vector.tensor_scalar_add(
                        qpad_sb[0:64, 0, s0:s0 + 256], ps[0:64, :],
                        qkb_sb[0:64, 0:1])
                    nc.vector.tensor_scalar_add(
                        qpad_sb[64:128, 1, s0:s0 + 256], ps[64:128, :],
                        qkb_sb[64:128, 0:1])
                return emit

            def k_unit(b, pp):
                """K projection for one t-pair (256 tokens) -> k_sb"""
                def emit():
                    uid[0] += 1
                    t0 = b * 2048 + pp * 256
                    ps = qk_ps.tile([128, 256], f32, tag="qk",
                                    name=f"k{uid[0]}")
                    for dk in range(8):
                        nc.tensor.matmul(
                            ps[:, :], wk_sb[:, dk, :],
                            q_sb[:, dk, t0:t0 + 256],
                            start=(dk == 0), stop=(dk == 7))
                    nc.vector.tensor_scalar_add(
                        k_sb[:, t0:t0 + 256], ps[:, :], qkb_sb[:, 1:2])
                return emit

            def v_unit(b, st):
                """V projection for one t-tile in [t, vdim] layout."""
                def emit():
                    uid[0] += 1
                    t0 = b * 2048 + st * 128
                    ps = wv_ps.tile([128, 128], f32, tag="wv",
                                    name=f"v{uid[0]}")
                    for dk in range(8):
                        nc.tensor.matmul(
                            ps[:, :], q_sb[:, dk, t0:t0 + 128],
                            wv_sb[:, dk, :],
                            start=(dk == 0), stop=False)
                    nc.tensor.matmul(  # += ones.T @ vb  (per-vdim bias)
                        ps[:, :], ones_sb[0:1, :], vb_sb[0:1, :],
                        start=False, stop=True)
                    # GPSIMD cannot access PSUM (BIR verifier); one
                    # strided DVE copy fills both head halves around the
                    # resident ones columns
                    nc.vector.tensor_copy(
                        v_sb[:, b, st, 0:130].rearrange(
                            "p (g c) -> p g c", g=2)[:, :, 0:64],
                        ps[:, :].rearrange("p (g c) -> p g c", g=2))
                return emit

            def proj_part(b, ot, off, w, eng=None):
                """partial[ot*128:, b*2048+off : +w] via stage copy."""
                def emit():
                    uid[0] += 1
                    ps = wv_ps.tile([128, w], f32, tag="wv",
                                    name=f"p{uid[0]}")
                    nc.tensor.matmul(
                        ps[:, :], w2_sb[:, ot * 128:(ot + 1) * 128],
                        outT_sb[:, b, off:off + w], start=True, stop=True)
                    stg = stage_pool.tile([128, w], f16, tag="st",
                                          name=f"s{uid[0]}")
                    (eng or nc.vector).tensor_copy(stg[:, :], ps[:, :])
                    nc.sync.dma_start(
                        out_d[ot * 128:(ot + 1) * 128,
                              b * 2048 + off:b * 2048 + off + w],
                        stg[:, :])
                return emit

            # ---- spine machinery ---------------------------------------
            pv_state = {}

            def emit_pv(e, b, j, p):
                st = pv_state[(b, j)]
                for sub in range(2):
                    step = 2 * p + sub
                    for k in range(2):
                        for h in range(2):
                            nc.tensor.matmul(
                                st["tile"][:, 2 * k + h, :],
                                e[:, sub, h, k * 128:(k + 1) * 128],
                                v_sb[:, b, 2 * p + sub, 65 * h:65 * h + 65],
                                start=(step == 0 and k == 0 and h == 0),
                                stop=(step == 15 and k == 1 and h == 1))

            def norm_block(b, j, split=False):
                def emit():
                    pvt = pv_state[(b, j)]["tile"]
                    rc = recip_pool.tile([128, 4, 1], f32, tag="rc",
                                         name=f"rc{b}_{j}")
                    nc.vector.reciprocal(rc[:, :, :], pvt[:, :, 64:65])
                    if split:
                        # tail only: k-subtiles in SEPARATE tiles so the
                        # DVE and ACT writers don't serialize on a tile WAW;
                        # ACT Copy with per-partition scale = normalize
                        ats = [attn_pool.tile([128, 1, 128], f16, tag="at",
                                              name=f"at{b}_{j}_{k}")
                               for k in range(2)]
                        pv_state[(b, j)]["attn"] = ats
                        for k in range(2):
                            for h in range(2):
                                if k == 1:
                                    nc.scalar.activation(
                                        ats[k][:, 0, 64 * h:64 * h + 64],
                                        pvt[:, 2 * k + h, 0:64],
                                        AF.Copy, scale=rc[:, 2 * k + h, 0:1])
                                else:
                                    nc.vector.tensor_scalar_mul(
                                        ats[k][:, 0, 64 * h:64 * h + 64],
                                        pvt[:, 2 * k + h, 0:64],
                                        rc[:, 2 * k + h, 0:1])
                        return
                    at = attn_pool.tile([128, 2, 128], f16, tag="at",
                                        name=f"at{b}_{j}")
                    pv_state[(b, j)]["attn"] = at
                    for k in range(2):
                        for h in range(2):
                            nc.vector.tensor_scalar_mul(
                                at[:, k, 64 * h:64 * h + 64],
                                pvt[:, 2 * k + h, 0:64],
                                rc[:, 2 * k + h, 0:1])
                return emit

            def transp_block(b, j, split=False):
                def emit():
                    at = pv_state[(b, j)]["attn"]
                    for k in range(2):
                        uid[0] += 1
                        src = at[k][:, 0, :] if split else at[:, k, :]
                        tr = wv_ps.tile([128, 128], f16, tag="wv",
                                        name=f"tr{uid[0]}")
                        nc.tensor.transpose(tr[:, :], src, ident_sb[:, :])
                        dst = outT_sb[:, b, j * 256 + k * 128:
                                      j * 256 + (k + 1) * 128]
                        if split and k == 1:
                            nc.scalar.activation(dst, tr[:, :], AF.Copy)
                        else:
                            nc.vector.tensor_copy(dst, tr[:, :])
                return emit

            # ---- weave schedule ---------------------------------------
            sched = {}

            def at(b, j, p, *units):
                sched.setdefault((b, j, p), []).extend(units)

            # b0 j0: stream b0's K and V against q-chunk arrivals; K units
            # lead their slot (scores are the critical consumer)
            at(0, 0, 0, k_unit(0, 1), v_unit(0, 0), v_unit(0, 1))
            at(0, 0, 1, k_unit(0, 2), k_unit(0, 3), v_unit(0, 2))
            at(0, 0, 2, qj_unit(0, 1), v_unit(0, 3), v_unit(0, 4))
            at(0, 0, 3, k_unit(0, 4), k_unit(0, 5), v_unit(0, 5))
            at(0, 0, 4, v_unit(0, 6), v_unit(0, 7), v_unit(0, 8))
            at(0, 0, 5, k_unit(0, 6), k_unit(0, 7), v_unit(0, 9))
            at(0, 0, 6, v_unit(0, 10), v_unit(0, 11), qj_unit(0, 2))
            at(0, 0, 7, v_unit(0, 12), v_unit(0, 13))
            # b0 j1..j7: rest of Q(b0), all QKV(b1) (placed after their
            # q-chunk DMA arrivals), proj(b0) filling the chunk-wait slack
            at(0, 1, 0, v_unit(0, 14))
            at(0, 1, 1, v_unit(0, 15))
            at(0, 1, 2, qj_unit(0, 3))
            at(0, 1, 3, v_unit(1, 0))
            at(0, 1, 4, k_unit(1, 0))
            at(0, 1, 5, v_unit(1, 1))
            at(0, 1, 6, k_unit(1, 1))
            at(0, 1, 7, qj_unit(0, 4))
            at(0, 2, 0, v_unit(1, 2))
            at(0, 2, 1, v_unit(1, 3))
            at(0, 2, 2, k_unit(1, 2))
            at(0, 2, 3, proj_part(0, 0, 0, 512), v_unit(1, 4))
            at(0, 2, 4, proj_part(0, 1, 0, 512))
            at(0, 2, 5, k_unit(1, 3))
            at(0, 2, 6, v_unit(1, 5))
            at(0, 2, 7, v_unit(1, 6))
            at(0, 3, 0, v_unit(1, 7))
            at(0, 3, 1, proj_part(0, 2, 0, 512))
            at(0, 3, 2, k_unit(1, 4))
            at(0, 3, 3, proj_part(0, 3, 0, 512))
            at(0, 3, 4, proj_part(0, 4, 0, 512))
            at(0, 3, 5, v_unit(1, 8))
            at(0, 3, 6, proj_part(0, 5, 0, 512))
            at(0, 3, 7, proj_part(0, 6, 0, 512))
            at(0, 4, 0, v_unit(1, 9))
            at(0, 4, 1, qj_unit(0, 5))
            at(0, 4, 2, k_unit(1, 5))
            at(0, 4, 3, v_unit(1, 10))
            at(0, 4, 4, proj_part(0, 7, 0, 512))
            at(0, 4, 5, proj_part(0, 0, 512, 512))
            at(0, 4, 6, v_unit(1, 11))
            at(0, 4, 7, k_unit(1, 6))
            at(0, 5, 0, v_unit(1, 12))
            at(0, 5, 1, proj_part(0, 1, 512, 512))
            at(0, 5, 2, v_unit(1, 13))
            at(0, 5, 3, proj_part(0, 2, 512, 512))
            at(0, 5, 4, qj_unit(0, 6))
            at(0, 5, 5, k_unit(1, 7))
            at(0, 5, 6, v_unit(1, 14))
            at(0, 5, 7, proj_part(0, 3, 512, 512))
            at(0, 6, 0, qj_unit(0, 7))
            at(0, 6, 1, v_unit(1, 15))
            at(0, 6, 2, proj_part(0, 4, 512, 512))
            at(0, 6, 3, proj_part(0, 5, 512, 512))
            at(0, 6, 4, qj_unit(1, 0))
            at(0, 6, 5, proj_part(0, 6, 512, 512))
            at(0, 6, 6, qj_unit(1, 1))
            at(0, 6, 7, proj_part(0, 7, 512, 512))
            at(0, 7, 0, qj_unit(1, 2))
            at(0, 7, 1, proj_part(0, 0, 1024, 512))
            at(0, 7, 2, proj_part(0, 1, 1024, 512))
            at(0, 7, 3, proj_part(0, 2, 1024, 512))
            at(0, 7, 4, proj_part(0, 3, 1024, 512))
            at(0, 7, 5, proj_part(0, 4, 1024, 512))
            at(0, 7, 6, proj_part(0, 5, 1024, 512))
            at(0, 7, 7, qj_unit(1, 3))
            # b1: rest of Q(b1), proj(b0 tail), all proj(b1)
            at(1, 0, 0, proj_part(0, 6, 1024, 512))
            at(1, 0, 1, proj_part(0, 7, 1024, 512))
            at(1, 0, 4, qj_unit(1, 4))
            at(1, 0, 5, proj_part(0, 0, 1536, 512))
            at(1, 0, 6, proj_part(0, 1, 1536, 512))
            at(1, 0, 7, proj_part(0, 2, 1536, 512))
            at(1, 1, 0, proj_part(0, 3, 1536, 512))
            at(1, 1, 1, proj_part(0, 4, 1536, 512))
            at(1, 1, 2, proj_part(0, 5, 1536, 512))
            at(1, 1, 3, proj_part(0, 6, 1536, 512))
            at(1, 1, 4, proj_part(0, 7, 1536, 512))
            at(1, 1, 5, qj_unit(1, 5))
            at(1, 2, 0, qj_unit(1, 6))
            at(1, 2, 1, qj_unit(1, 7))
            for ot in range(8):
                at(1, 3, ot, proj_part(1, ot, 0, 512))
            for ot in range(8):
                at(1, 5, ot, proj_part(1, ot, 512, 512))
            for i, ot in enumerate(range(5)):
                at(1, 6, 3 + i, proj_part(1, ot, 1024, 512))
            at(1, 7, 0, proj_part(1, 5, 1024, 512))
            at(1, 7, 1, proj_part(1, 6, 1024, 512))
            at(1, 7, 2, proj_part(1, 7, 1024, 512))
            at(1, 7, 3, proj_part(1, 0, 1536, 256),
               proj_part(1, 1, 1536, 256),
               proj_part(1, 2, 1536, 256))
            at(1, 7, 4, proj_part(1, 3, 1536, 256),
               proj_part(1, 4, 1536, 256),
               proj_part(1, 5, 1536, 256))
            at(1, 7, 5, proj_part(1, 6, 1536, 256),
               proj_part(1, 7, 1536, 256))

            deferred = {}  # (b, j, p) -> [callables]

            # ---- phase 1: warmup + minimal pre-work --------------------
            wps = wv_ps.tile([128, 128], f32, tag="wv", name="wps")
            for i in range(37):
                nc.tensor.matmul(wps[:, :], wlhs_sb[:, :], wlhs_sb[:, :],
                                 start=True, stop=True)
            # pre-phase Q goes through the wv_ps bank so K's matmuls don't
            # serialize behind Q's psum->sbuf copy (qk_ps is single-buffered)
            qj_unit(0, 0, pool=wv_ps)()
            k_unit(0, 0)()

            # ---- attention spine ---------------------------------------
            # pv runs TWO slots behind exp so a block's last pv never
            # catches up with its exp at the j boundary
            e_queue = []
            nblocks = 0
            for b in range(B):
                for j in range(JB):
                    if _TRUNC is not None and nblocks >= _TRUNC:
                        continue
                    nblocks += 1
                    pv_state[(b, j)] = {
                        "tile": pv_pool.tile([128, 4, 65], f32, tag="pv",
                                             name=f"pv{b}_{j}")}
                    for p in range(PP):
                        s0 = b * 2048 + j * 256
                        sc = sc_pool.tile([128, 2, 2, 256], f32, tag="sc",
                                          name=f"sc{b}_{j}_{p}")
                        if "sc" not in _SKIP:
                            # one accumulation group per bank: h0 starts
                            # (pending-zeroes the bank), h1's first write
                            # replaces its half and stops the group
                            for sub in range(2):
                                t0 = b * 2048 + (2 * p + sub) * 128
                                for h in range(2):
                                    nc.tensor.matmul(
                                        sc[:, sub, h, :],
                                        k_sb[:, t0:t0 + 128],
                                        qpad_sb[:, h, s0:s0 + 256],
                                        start=(h == 0), stop=(h == 1))
                        for u in deferred.pop((b, j, p), ()):
                            u()
                        if "weave" not in _SKIP:
                            for u in sched.pop((b, j, p), ()):
                                u()
                        if len(e_queue) >= 2:
                            emit_pv(*e_queue.pop(0))
                        if (b, j) == (1, 7) and p == 7 and e_queue:
                            emit_pv(*e_queue.pop(0))
                        if "exp" in _SKIP:
                            continue
                        e = exp_pool.tile([128, 2, 2, 256], f16, tag="e",
                                          name=f"e{b}_{j}_{p}")
                        nc.scalar.activation(e[:, :, :, :], sc[:, :, :, :],
                                             AF.Exp, scale=0.125)
                        if "pv" not in _SKIP:
                            e_queue.append((e, b, j, p))
                    # norm/transpose of j ride block j+1's slots 2/3
                    # (pv(j, p7) lands at slot 1 via the lag-2 queue)
                    if (b, j) != (1, 7):
                        nb, nj = (b, j + 1) if j < 7 else (b + 1, 0)
                        deferred.setdefault((nb, nj, 2), []).append(
                            norm_block(b, j))
                        deferred.setdefault((nb, nj, 3), []).append(
                            transp_block(b, j))

            if _TRUNC is not None:
                # debug truncation: drain pv chains, skip norm/tail, dump
                # qk_sb and v_sb for host inspection
                for item in e_queue:
                    emit_pv(*item)
                e_queue = []
                nc.sync.dma_start(out_d[0:128, :], qpad_sb[:, 0, :])
                nc.sync.dma_start(out_d[128:256, :], k_sb[:, :])
                nc.sync.dma_start(
                    out_d[256:384, 0:16 * 130],
                    v_sb[:, 0, :, :])
                nc.sync.dma_start(out_d[384:512, :], q_sb[:, 0, :])
                nc.sync.dma_start(out_d[512:640, 0:1024],
                                  wq_sb[:, :, :])
            if _TRUNC is None:
                assert not sched, f"unconsumed weave slots: {list(sched)}"

                # ---- tail: drain the lag queue, norm/transpose j7, project
                # the final 256 cols with wide stage copies ----
                for item in e_queue:
                    emit_pv(*item)
                norm_block(1, 7, split=True)()
                transp_block(1, 7, split=True)()
                out_t = out_d.rearrange("(a p) m -> p a m", p=128)
                for half in range(2):
                    tps = sc_pool.tile([128, 2, 2, 256], f32, tag="sc",
                                       name=f"tail{half}")
                    for i in range(4):
                        ot = half * 4 + i
                        nc.tensor.matmul(
                            tps[:, i // 2, i % 2, :],
                            w2_sb[:, ot * 128:(ot + 1) * 128],
                            outT_sb[:, 1, 1792:2048], start=True, stop=True)
                    stg = stage_pool.tile([128, 4, 256], f16, tag="tt",
                                          name=f"tstg{half}")
                    if half == 0:
                        nc.vector.tensor_copy(stg[:, :, :], tps[:, :, :, :])
                    else:
                        nc.scalar.activation(stg[:, :, :], tps[:, :, :, :],
                                             AF.Copy)
                    nc.sync.dma_start(
                        out_t[:, half * 4:(half + 1) * 4, 3840:4096],
                        stg[:, :, :])
    nc.compile()
    return nc


def _get_nc():
    if "nc" not in _COMPILED:
        _COMPILED["nc"] = _build()
    return _COMPILED["nc"]


def _prep_inputs(q, in_w, qkv_bias):
    f16 = np.float16
    qT = np.ascontiguousarray(q.transpose(2, 0, 1).reshape(D, BS)).astype(f16)
    def pre(w):  # [128 odims, 1024 d] -> [128 part, 8 dk, 128 odims]
        return np.ascontiguousarray(
            w.T.reshape(8, 128, 128).transpose(1, 0, 2)).astype(f16)

    maps = []
    for c in range(NCORES):
        r = slice(128 * c, 128 * (c + 1))
        wq, wk, wv = in_w[0:D][r], in_w[D:2 * D][r], in_w[2 * D:3 * D][r]
        maps.append({
            "qT": qT,
            "wq": pre(wq),
            "wk": pre(wk),
            "wv": pre(wv),
            "w2": None,  # filled with out_w slice
            "qkb": np.ascontiguousarray(
                np.stack([qkv_bias[0:D][r], qkv_bias[D:2 * D][r]], axis=1)
            ).astype(np.float32),
            "vb": np.ascontiguousarray(
                qkv_bias[2 * D:3 * D][r][None, :]).astype(f16),
        })
    return maps


def kernel(q, k, v, in_w, qkv_bias, out_w, out_b, _trace=False):
    from concourse.bass_utils import run_bass_kernel_spmd

    q = np.asarray(q, dtype=np.float32)
    in_w = np.asarray(in_w, dtype=np.float32)
    qkv_bias = np.asarray(qkv_bias, dtype=np.float32)
    out_w = np.asarray(out_w, dtype=np.float32)
    out_b = np.asarray(out_b, dtype=np.float32)

    nc = _get_nc()
    in_maps = _prep_inputs(q, in_w, qkv_bias)
    for c in range(NCORES):
        r = slice(128 * c, 128 * (c + 1))
        in_maps[c]["w2"] = np.ascontiguousarray(out_w[:, r].T).astype(np.float16)

    res = run_bass_kernel_spmd(
        nc, in_maps, core_ids=list(range(NCORES)), trace=_trace,
    )
    total = np.zeros((D, BS), dtype=np.float32)
    for c in range(NCORES):
        total += res.results[c]["partial"].astype(np.float32)
    net = total.T + out_b[None, :]
    out = net.reshape(B, S, D).astype(np.float32)
    if _trace:
        return out, res
    return out
